# revision 37
# baseline (speedup 1.0000x reference)
"""GNN attention layer (nn_Attention_Layer_21131239096479) on 8 TRN2 NeuronCores.

v3 design (edge/dst parallel, LayerNorm algebraically decomposed):
 - key[e] = inv_sig[e]*(ea[e]@A_k + XBk[src] + XCk[dst]) - mu[e]*inv_sig[e]*s_k + c_k
   with mu/sig from per-node/per-edge running sums; same for value.  Softmax
   denominators applied post-scatter so one pass over edges suffices.
 - Nodes are degree-balance-binned into 8 cores x 10 windows of <=128 dst
   nodes so every window carries ~4000 edges (W_E=4096, superblocks of 512).
 - Per-node tables XBk|XBv live in an FP8(e4m3) DRAM table (TROW=512) built
   on-device (phase A, fp8 DoubleRow matmuls + scale-compensated quant) and
   fetched per-edge with dma_gather (512-row calls, 2 swdge queues).
 - Per-edge LN stats host-pregathered (ea stats + sx[src]); one-hot gather/
   scatter matrices PREBUILT ON HOST (no on-device is_equal).
 - Core fusion: ea[e]@[A_k|A_v] + XB[src] lands in ONE fp8 DoubleRow matmul
   per 128-edge group (contraction subtile 0 = identity x gathered GT rows,
   subtile 1 = ea^T/8 x 8*akv stored as slot 8 of the gather tile).
 - q + per-dst-dot gathers ride one OHD stationary; value scatter streams
   msg|softmax-stat columns through a single PSUM accumulation chain
   (interleaved psum chains in one bank are BROKEN on this HW - keep one).
 - (c,h) column interleave everywhere so per-(edge,head) scalars broadcast
   along the packed last dim (DVE 2x); prod written (h,c)-packed so the
   per-head reduce runs on a packed axis.
 - LN-stat/softmax chain batched per window (1-iter Newton rsqrt, clamped
   exp, no segment-max pass); MLP runs as a stage-interleaved tail in
   2-window groups (engines execute streams in PROGRAM ORDER - interleaving
   stages across windows keeps every queue fed).
"""
import math
import numpy as np
from contextlib import ExitStack

import concourse.bass as bass
import concourse.bacc as bacc
import concourse.mybir as mybir
import concourse.tile as tile
import concourse.bass_utils as bass_utils
from concourse import library_config
import ml_dtypes

FP32 = mybir.dt.float32
BF16 = mybir.dt.bfloat16
F8E4 = mybir.dt.float8e4
PM = mybir.MatmulPerfMode
I32 = mybir.dt.int32
I16 = mybir.dt.int16
AF = mybir.ActivationFunctionType
ALU = mybir.AluOpType
AX = mybir.AxisListType

N, E = 10000, 320000
CZ, CE, CO, H, CF = 256, 64, 32, 8, 576
NCORES, NWIN = 8, 10
NBIN = NCORES * NWIN
NLOC_PAD = NWIN * 128          # 1280 window-slot rows per core
SBE = 512                      # edges per superblock
NPAD = 10112                   # padded node-table rows (79*128)
NTB = NPAD // 128              # 79
TROW = 512                     # fp8 table row: XBk(256) XBv(256)
BF = ml_dtypes.bfloat16
E4 = ml_dtypes.float8_e4m3

# (c,h) interleave permutation: new col c*8+h <- old col h*32+c
PERM_CH = np.arange(H * CO).reshape(H, CO).T.reshape(-1)


class Cfg:
    def __init__(self, **kw):
        self.__dict__.update(kw)


def _balance_bins(deg):
    """Greedy degree-balanced assignment of N nodes into NBIN bins (<=128 each).
    Returns binof[n], slot[n]."""
    import heapq
    order = np.argsort(-deg, kind="stable")
    binof = np.zeros(N, np.int32)
    slot = np.zeros(N, np.int32)
    heap = [(0, 0, b) for b in range(NBIN)]
    heapq.heapify(heap)
    stash = []
    for n in order:
        while True:
            load, cnt, b = heapq.heappop(heap)
            if cnt < 128:
                break
            stash.append((load, cnt, b))
        for s in stash:
            heapq.heappush(heap, s)
        stash.clear()
        binof[n] = b
        slot[n] = cnt
        heapq.heappush(heap, (load + int(deg[n]), cnt + 1, b))
    return binof, slot


def _host_prep(cfg, x, edge_index, edge_attr, ln_gamma, ln_beta, Wq, bq, Wk, bk, Wv, bv,
               W1, b1, W2, b2):
    f32 = np.float32
    P = PERM_CH
    x = np.asarray(x, f32)
    ei = np.asarray(edge_index)
    ea = np.asarray(edge_attr, f32)
    gamma = np.asarray(ln_gamma, f32); beta = np.asarray(ln_beta, f32)
    Wq = np.asarray(Wq, f32)[:, P] / math.sqrt(CO)
    bq = np.asarray(bq, f32)[P] / math.sqrt(CO)
    Wk = np.asarray(Wk, f32)[:, P]; bk = np.asarray(bk, f32)[P]
    Wv = np.asarray(Wv, f32)[:, P]; bv = np.asarray(bv, f32)[P]
    W1 = np.asarray(W1, f32)[P, :]; b1 = np.asarray(b1, f32)
    W2 = np.asarray(W2, f32); b2 = np.asarray(b2, f32)

    Wkg = Wk * gamma[:, None]; Wvg = Wv * gamma[:, None]
    A_k, B_k, C_k = Wkg[:CE], Wkg[CE:CE + CZ], Wkg[CE + CZ:]
    A_v, B_v, C_v = Wvg[:CE], Wvg[CE:CE + CZ], Wvg[CE + CZ:]
    s_k = Wkg.sum(0); c_k = beta @ Wk + bk
    s_v = Wvg.sum(0); c_v = beta @ Wv + bv

    def chunk_pack(M, kchunks):
        K, Nc = M.shape
        assert K == kchunks * 128
        return np.ascontiguousarray(M.reshape(kchunks, 128, Nc).transpose(1, 0, 2))

    akv8 = np.zeros((128, 512), E4)
    akv8[:64] = (np.concatenate([A_k, A_v], 1) * 8.0).astype(E4)
    consts = {
        "akv8": akv8,
        "wqck": chunk_pack(np.concatenate([Wq, C_k], 1), 2).astype(BF),    # [128,2,512]
        "cv": chunk_pack(C_v, 2).astype(BF),                               # [128,2,256]
        "bkv": chunk_pack(np.concatenate([B_k, B_v], 1) * 16.0, 2).astype(E4),  # [128,2,512]
        "w1": chunk_pack(W1, 2).astype(BF),                                # [128,2,512]
        "w2": chunk_pack(W2, 4).astype(BF),                                # [128,4,256]
        "skb": np.tile(s_k, (128, 1)).astype(BF),
        "ckb": np.tile(c_k, (128, 1)).astype(BF),
        "svb": np.tile(s_v, (128, 1)).astype(f32),
        "cvb": np.tile(c_v, (128, 1)).astype(f32),
        "bqb": np.tile(bq, (128, 1)).astype(f32),
        "b1b": np.tile(b1, (128, 1)).astype(f32),
        "b2b": np.tile(b2, (128, 1)).astype(f32),
        "identf": np.eye(128, dtype=f32),
    }

    # ---- node binning (degree balanced) ----
    src, dst = ei[0].astype(np.int64), ei[1].astype(np.int64)
    deg = np.bincount(dst, minlength=N)
    binof, slot = _balance_bins(deg)
    core_of = binof // NWIN
    win_of = binof % NWIN

    # node stats
    sx = x.sum(1)                     # [N]
    sqx = (x * x).sum(1)

    x_pad = np.zeros((NPAD, CZ), f32); x_pad[:N] = x

    # transposed x for phase A: xpt[p, (b*2+k)*128+m] = x_pad[b*128+m, k*128+p]
    # fp8 with scale compensation: (x/2) @ (16*B) = 8*XB; table copy scales by 1/8
    xpt = np.ascontiguousarray(
        (x_pad / 2.0).reshape(NTB, 128, 2, 128).transpose(3, 0, 2, 1)).astype(E4)
    xpt = xpt.reshape(128, NTB * 2 * 128)

    # per-core edge grouping
    ecore = core_of[dst]
    ewin = win_of[dst]
    eslot = slot[dst]
    # per-edge LN stat pre-sums: edge_attr part + src-node part (host gather)
    ep_sum = (ea.sum(1) + sx[src]).astype(f32)
    ep_sqs = ((ea * ea).sum(1) + sqx[src]).astype(f32)

    maxcnt = 0
    per_core = []
    for c in range(NCORES):
        m = ecore == c
        esrc = src[m]; ew = ewin[m]; es = eslot[m]; eidx = np.nonzero(m)[0]
        order = np.argsort(ew, kind="stable")
        esrc, ew, es, eidx = esrc[order], ew[order], es[order], eidx[order]
        counts = np.bincount(ew, minlength=NWIN)
        maxcnt = max(maxcnt, int(counts.max()))
        per_core.append((esrc, ew, es, eidx, counts))

    W_E = int(math.ceil(maxcnt / SBE) * SBE)
    NSB = W_E // SBE
    NSBT = NWIN * NSB
    NG = NSBT * 4                  # total 128-edge groups per core
    NPAIR = NSBT // 2              # 1024-edge gather calls per core

    in_maps = []
    for c in range(NCORES):
        esrc, ew, es, eidx, counts = per_core[c]
        idx16 = np.full(NWIN * W_E, -1, np.int16)
        drel = np.full(NWIN * W_E, -1, np.int32)
        ea_t = np.zeros((CE, NWIN * W_E), f32)
        easum = np.zeros((NWIN * W_E, 2), f32)
        pos = 0
        for wi in range(NWIN):
            cnt = int(counts[wi])
            s = wi * W_E
            sl = slice(pos, pos + cnt)
            idx16[s:s + cnt] = esrc[sl]
            drel[s:s + cnt] = es[sl]
            ea_t[:, s:s + cnt] = ea[eidx[sl]].T
            easum[s:s + cnt, 0] = ep_sum[eidx[sl]]
            easum[s:s + cnt, 1] = ep_sqs[eidx[sl]]
            pos += cnt
        # DoubleRow stationary: per window 33 slots of [128, 128] fp8:
        # slot 0 = identity, slot 1+g = [ea^T/8 (64 rows); zeros]
        NSBW = W_E // SBE
        NGW = NSBW * 4
        EAD = np.zeros((128, NWIN, NGW + 1, 128), E4)
        ident8 = np.eye(128).astype(E4)
        for wi in range(NWIN):
            EAD[:, wi, 0, :] = ident8
            blk8 = (ea_t[:, wi * W_E:(wi + 1) * W_E] / 8.0).astype(E4)
            EAD[:64, wi, 1:, :] = blk8.reshape(64, NGW, 128)
        EAD = np.ascontiguousarray(EAD).reshape(128, NWIN * (NGW + 1) * 128)
        # gather index layout: 512-idx calls, idx j -> [j % 16, j // 16]
        IDX = np.full((128, NSBT * 32), -1, np.int16)
        blk = idx16.reshape(NSBT, 32, 16)
        IDX[:16] = blk.transpose(2, 0, 1).reshape(16, NSBT * 32)
        IDX[16:] = np.tile(IDX[:16], (7, 1))
        # host-prebuilt one-hot matrices
        valid = drel >= 0
        epos = np.nonzero(valid)[0]
        dslot = drel[valid]
        # OHD[p, e] = (dslot[e] == p)  -- partition = dst slot, free = edge
        OHD = np.zeros((128, NWIN * W_E), BF)
        OHD[dslot, epos] = 1.0
        # OHE[p, g*128 + d] = 1 where p = edge-in-group, g = global group
        OHE = np.zeros((128, NG * 128), BF)
        OHE[epos % 128, (epos // 128) * 128 + dslot] = 1.0
        # edge-major per-edge stat pre-sums [128, NG, 2]
        EAS = np.ascontiguousarray(
            easum.reshape(NG, 128, 2).transpose(1, 0, 2)
        ).astype(BF).reshape(128, NG * 2)

        # window-local node features (permuted), transposed + stats
        nodes = np.nonzero(core_of == c)[0]
        x_loc = np.zeros((NLOC_PAD, CZ), f32)
        sxq_loc = np.zeros((NLOC_PAD, 2), f32)
        rows = win_of[nodes] * 128 + slot[nodes]
        x_loc[rows] = x[nodes]
        sxq_loc[rows, 0] = sx[nodes]; sxq_loc[rows, 1] = sqx[nodes]
        xlt = np.ascontiguousarray(
            x_loc.reshape(NWIN, 128, 2, 128).transpose(3, 0, 2, 1)).astype(BF)
        xlt = xlt.reshape(128, NWIN * 2 * 128)
        sxql = np.ascontiguousarray(
            sxq_loc.reshape(NWIN, 128, 2).transpose(1, 0, 2)).reshape(128, NWIN * 2)

        in_maps.append({
            "xpt": xpt,
            "xlt": xlt, "sxql": sxql.astype(f32),
            "ead": EAD,
            "idx": IDX, "ohd": OHD, "ohe": OHE, "easum": EAS,
        })

    # output unshard map: full[n] = per_core[core_of[n]][win*128+slot]
    unshard = (core_of, win_of * 128 + slot)
    return consts, in_maps, W_E, NSB, unshard


def _build(nc, tc, ctx, consts_h, cfg):
    NWIN_, NSB, W_E = cfg.NWIN, cfg.NSB, cfg.W_E
    NSBT = NWIN_ * NSB
    NG = NSBT * 4
    NPAIRW = NSB // 2              # 1024-edge pairs per window

    xpt_d = nc.dram_tensor("xpt", [128, NTB * 2 * 128], F8E4, kind="ExternalInput").ap()
    xlt_d = nc.dram_tensor("xlt", [128, NWIN * 2 * 128], BF16, kind="ExternalInput").ap()
    sxql_d = nc.dram_tensor("sxql", [128, NWIN * 2], FP32, kind="ExternalInput").ap()
    ead_d = nc.dram_tensor("ead", [128, NWIN * (NSB * 4 + 1) * 128], F8E4, kind="ExternalInput").ap()
    idx_d = nc.dram_tensor("idx", [128, NSBT * 32], I16, kind="ExternalInput").ap()
    ohd_d = nc.dram_tensor("ohd", [128, NWIN * W_E], BF16, kind="ExternalInput").ap()
    ohe_d = nc.dram_tensor("ohe", [128, NG * 128], BF16, kind="ExternalInput").ap()
    easum_d = nc.dram_tensor("easum", [128, NG * 2], BF16, kind="ExternalInput").ap()
    y_d = nc.dram_tensor("y", [NLOC_PAD, CZ], FP32, kind="ExternalOutput").ap()
    tsrc = nc.dram_tensor("tsrc", [NPAD, TROW], F8E4, kind="Internal").ap()

    cd = {k: nc.inline_tensor(np.asarray(v), name=f"c_{k}").ap() for k, v in consts_h.items()}
    nc.gpsimd.load_library(library_config.mlp)

    # ---------------- resident constants ----------------
    cpool = ctx.enter_context(tc.tile_pool(name="consts", bufs=1))
    cs = {}
    akv8_d = cd.pop("akv8")
    for k, ap in cd.items():
        ctile = cpool.tile(list(ap.shape), ap.dtype, tag=f"c_{k}")
        cs[k] = ctile
    nc.sync.dma_start(cs["bkv"][:], cd["bkv"])
    eas_sb = cpool.tile([128, NG, 2], BF16, tag="eassb")

    def load_rest_consts():
        for k, ap in cd.items():
            if k == "bkv":
                continue
            nc.sync.dma_start(cs[k][:], ap)
        nc.sync.dma_start(eas_sb[:], easum_d)

    # ---------------- pools ----------------
    # PSUM (8 banks): p_ea 2x2 + p_q 2x1 + p_qd 1 + p_sq 1
    p_ea = ctx.enter_context(tc.tile_pool(name="p_ea", bufs=2, space="PSUM"))
    p_q = ctx.enter_context(tc.tile_pool(name="p_q", bufs=2, space="PSUM"))
    p_qd = ctx.enter_context(tc.tile_pool(name="p_qd", bufs=1, space="PSUM"))
    p_sq = ctx.enter_context(tc.tile_pool(name="p_sq", bufs=1, space="PSUM"))

    sb_tab = ctx.enter_context(tc.tile_pool(name="sb_tab", bufs=2))
    sb_gt = ctx.enter_context(tc.tile_pool(name="sb_gt", bufs=3))
    sb_ea = ctx.enter_context(tc.tile_pool(name="sb_ea", bufs=2))
    sb_oh = ctx.enter_context(tc.tile_pool(name="sb_oh", bufs=2))
    sb_s1 = ctx.enter_context(tc.tile_pool(name="sb_s1", bufs=2))
    sb_mlp = ctx.enter_context(tc.tile_pool(name="sb_mlp", bufs=3))
    sb_win = ctx.enter_context(tc.tile_pool(name="sb_win", bufs=2))
    sb_att = ctx.enter_context(tc.tile_pool(name="sb_att", bufs=1))

    # ================= phase A: build src table (2 blocks per step) =========
    nba = (NTB + 3) // 4
    for bb in range(nba):
        j0 = bb * 4
        nj = min(4, NTB - j0)
        xb = sb_tab.tile([128, 4, 2, 128], F8E4, tag="xb")
        nc.sync.dma_start(xb[:, 0:nj, :, :], xpt_d[:, j0 * 256:(j0 + nj) * 256])
        to = sb_tab.tile([128, 4, TROW], F8E4, tag="to")
        njh = min(2, nj)
        mm = p_ea.tile([128, 2, 512], FP32, tag="ea")
        for j in range(njh):
            nc.tensor.matmul(mm[:, j, :], xb[:, j, :, :], cs["bkv"][:],
                             start=True, stop=True, perf_mode=PM.DoubleRow)
        nc.scalar.activation(to[:, 0:njh, :], mm[:, 0:njh, :], AF.Copy, scale=0.125)
        for j in range(2, nj):
            mq = p_q.tile([128, 2, 256], FP32, tag="q")
            mqf = mq[:].rearrange("p a b -> p (a b)")
            nc.tensor.matmul(mqf, xb[:, j, :, :], cs["bkv"][:],
                             start=True, stop=True, perf_mode=PM.DoubleRow)
            nc.vector.tensor_scalar(to[:, j, :], mqf, 0.125, None, ALU.mult)
        dst = tsrc[j0 * 128:(j0 + nj) * 128, :].rearrange("(j p) c -> p j c", p=128)
        nc.sync.dma_start(dst, to[:, 0:nj, :])

    load_rest_consts()

    # zero-init gather buffers (pad edges may read them before first fill)
    for _ in range(3):
        g0 = sb_gt.tile([128, 9, TROW], F8E4, tag="GT")
        nc.gpsimd.memset(g0[:], 0.0)

    attbuf = sb_att.tile([128, NWIN_, CZ], FP32, tag="attbuf")
    # ================= windows =================
    for w in range(NWIN_):
        # ---- window prep ----
        xw = sb_win.tile([128, 2, 128], BF16, tag="xw")
        nc.sync.dma_start(xw[:], xlt_d[:, w * 256:(w + 1) * 256])
        qxt = p_ea.tile([128, 2, 512], FP32, tag="ea")
        qx = qxt[:, 0, :]
        for k in range(2):
            nc.tensor.matmul(qx, xw[:, k, :], cs["wqck"][:, k, :],
                             start=(k == 0), stop=(k == 1))
        for k in range(2):
            nc.tensor.matmul(qxt[:, 1, 0:256], xw[:, k, :], cs["cv"][:, k, :],
                             start=(k == 0), stop=(k == 1))
        G = sb_win.tile([128, 282], BF16, tag="G")
        nc.vector.tensor_add(G[:, 0:256], qx[:, 0:256], cs["bqb"][:])
        xck = sb_win.tile([128, CZ], BF16, tag="xck")
        nc.scalar.copy(xck[:], qx[:, 256:512])
        dcol = sb_win.tile([128, 26], FP32, tag="dcol")
        scr = sb_win.tile([128, 3, CZ], BF16, tag="scrw")
        nc.vector.tensor_mul(scr[:, 0, :], G[:, 0:256], xck[:])
        nc.vector.tensor_mul(scr[:, 1, :], G[:, 0:256], cs["skb"][:])
        nc.vector.tensor_mul(scr[:, 2, :], G[:, 0:256], cs["ckb"][:])
        nc.vector.tensor_reduce(dcol[:, 0:24].rearrange("p (t h) -> p t h", h=8),
                                scr[:].rearrange("p t (c h) -> p t h c", h=8),
                                AX.X, ALU.add)
        nc.sync.dma_start(dcol[:, 24:26], sxql_d[:, w * 2:(w + 1) * 2])
        nc.scalar.copy(G[:, 256:282], dcol[:])
        xcv = sb_win.tile([128, CZ], FP32, tag="xcv")
        nc.scalar.copy(xcv[:], qxt[:, 1, 0:256])

        nslot = NSB * 4 + 1
        eawt = sb_ea.tile([128, nslot, 128], F8E4, tag="eaw")
        nc.sync.dma_start(eawt[:], ead_d[:, w * nslot * 128:(w + 1) * nslot * 128])
        idxw = sb_win.tile([128, NSB * 32], I16, tag="idxw")
        nc.sync.dma_start(idxw[:], idx_d[:, w * NSB * 32:(w + 1) * NSB * 32])
        ohd = sb_oh.tile([128, W_E], BF16, tag="ohd")
        nc.sync.dma_start(ohd[:], ohd_d[:, w * W_E:(w + 1) * W_E])
        ohe = sb_oh.tile([128, NSB * 4, 128], BF16, tag="ohe")
        nc.sync.dma_start(ohe[:], ohe_d[:, w * NSB * 4 * 128:(w + 1) * NSB * 4 * 128])

        kvv = sb_win.tile([128, NSB * 4, 256], BF16, tag="kvv")
        winlc = sb_win.tile([128, NSB * 4, 8], FP32, tag="winlc")
        windc = sb_win.tile([128, NSB * 4, 24], BF16, tag="windc")
        wstp = sb_win.tile([128, NSB * 4, 2], FP32, tag="wstp")
        wst2 = sb_win.tile([128, NSB * 4, 4], FP32, tag="wst2")
        winU = sb_win.tile([128, NSB * 4, 24], BF16, tag="winU")
        lg = sb_win.tile([128, NSB * 4, 8], FP32, tag="lg")

        scat = p_sq.tile([128, 280], FP32, tag="sq")

        # ---- loop1 over superblock pairs (1024 edges per gather) ----
        for gg in range(NPAIRW):
            s8 = slice(gg * 8, (gg + 1) * 8)
            GT = sb_gt.tile([128, 9, TROW], F8E4, tag="GT")
            for hh in range(2):
                nc.gpsimd.dma_gather(GT[:, hh * 4:(hh + 1) * 4, :], tsrc,
                                     idxw[:, (gg * 2 + hh) * 32:(gg * 2 + hh + 1) * 32],
                                     SBE, SBE, TROW, queue_num=hh)
            nc.sync.dma_start(GT[:, 8, :], akv8_d)
            prod = sb_s1.tile([128, 8, 8, CO], BF16, tag="prod")
            qs = sb_s1.tile([128, 8, 256], BF16, tag="qs")
            qd = p_qd.tile([128, 8, 26], FP32, tag="qd")
            for t in range(4):
                kvp = p_ea.tile([128, 2, 512], FP32, tag="ea")
                qgp = p_q.tile([128, 2, 256], FP32, tag="q")
                for j2 in range(2):
                    jj = t * 2 + j2
                    eg = (gg * 8 + jj) * 128
                    sl = 1 + gg * 8 + jj
                    nc.tensor.matmul(kvp[:, j2, :],
                                     eawt[:, 0:sl + 1:sl, :],
                                     GT[:, jj:9:(8 - jj), :],
                                     start=True, stop=True, perf_mode=PM.DoubleRow)
                    ohd_s = ohd[:, eg:eg + 128]
                    nc.tensor.matmul(qgp[:, j2, :], ohd_s, G[:, 0:256])
                    nc.tensor.matmul(qd[:, jj, :], ohd_s, G[:, 256:282])
                s2 = slice(t * 2, (t + 1) * 2)
                nc.scalar.copy(qs[:, s2, :], qgp[:])
                # prod written (h,c)-packed so the reduce runs on a packed axis
                nc.vector.tensor_mul(
                    prod[:, s2, :, :],
                    kvp[:, :, 0:256].rearrange("p s (c h) -> p s h c", h=8),
                    qs[:, s2, :].rearrange("p s (c h) -> p s h c", h=8))
                nc.scalar.copy(kvv[:, gg * 8 + t * 2:gg * 8 + (t + 1) * 2, :],
                               kvp[:, :, 256:512])
            nc.vector.tensor_reduce(winlc[:, s8, :], prod[:], AX.X, ALU.add)
            nc.vector.tensor_add(wstp[:, s8, :], qd[:, :, 24:26],
                                 eas_sb[:, w * NSB * 4 + gg * 8:w * NSB * 4 + (gg + 1) * 8, :])
            nc.scalar.copy(windc[:, s8, :], qd[:, :, 0:24])

        # ---- batched LN-stat + softmax chain ----
        mu = wst2[:, :, 0:1]; var = wst2[:, :, 1:2]
        inv = wst2[:, :, 2:3]; muinv = wst2[:, :, 3:4]
        nc.scalar.activation(mu, wstp[:, :, 0:1], AF.Copy, scale=1.0 / CF)
        nc.scalar.activation(var, wstp[:, :, 1:2], AF.Copy, bias=1e-5, scale=1.0 / CF)
        nscr = sb_win.tile([128, NSB * 4, 1], FP32, tag="nscr")
        nc.vector.tensor_mul(nscr[:], mu, mu)
        nc.vector.tensor_sub(var, var, nscr[:])
        nc.vector.tensor_scalar(var, var, 1e-5, None, ALU.max)
        # Newton rsqrt, y0 = 1 (var is ~1 +- 0.1): 1 iteration
        nc.vector.tensor_scalar(inv, var, -0.5, 1.5, ALU.mult, ALU.add)
        for _ in range(1):
            nc.vector.tensor_mul(nscr[:], var, inv)
            nc.vector.tensor_mul(nscr[:], nscr[:], inv)
            nc.vector.tensor_scalar(nscr[:], nscr[:], -0.5, 1.5, ALU.mult, ALU.add)
            nc.vector.tensor_mul(inv, inv, nscr[:])
        nc.vector.tensor_mul(muinv, mu, inv)
        # logits
        inv_b = inv.broadcast_to([128, NSB * 4, 8])
        muinv_b = muinv.broadcast_to([128, NSB * 4, 8])
        lg8 = sb_win.tile([128, NSB * 4, 8], FP32, tag="lg8")
        nc.vector.tensor_add(lg[:], winlc[:], windc[:, :, 0:8])
        nc.vector.tensor_mul(lg[:], lg[:], inv_b)
        nc.vector.tensor_mul(lg8[:], windc[:, :, 8:16], muinv_b)
        nc.vector.tensor_sub(lg[:], lg[:], lg8[:])
        nc.vector.tensor_add(lg[:], lg[:], windc[:, :, 16:24])
        nc.vector.tensor_scalar(lg[:], lg[:], 15.0, None, ALU.min)
        nc.scalar.activation(winU[:, :, 0:8], lg[:], AF.Exp)
        nc.vector.tensor_mul(winU[:, :, 8:16], winU[:, :, 0:8], inv_b)
        nc.vector.tensor_mul(winU[:, :, 16:24], winU[:, :, 8:16],
                             mu.broadcast_to([128, NSB * 4, 8]))

        # ---- loop2: value messages + scatter ----
        for gg in range(NPAIRW):
            s8 = slice(gg * 8, (gg + 1) * 8)
            msg = sb_s1.tile([128, 8, 280], BF16, tag="msg")
            nc.vector.tensor_mul(
                msg[:, :, 0:256].rearrange("p s (c h) -> p s c h", h=8),
                kvv[:, s8, :].rearrange("p s (c h) -> p s c h", h=8),
                winU[:, s8, 8:16].unsqueeze(2).broadcast_to([128, 8, CO, 8]))
            nc.scalar.copy(msg[:, :, 256:280], winU[:, s8, :])
            for j in range(8):
                st = ohe[:, gg * 8 + j, :]
                first = (gg == 0 and j == 0)
                last = (gg == NPAIRW - 1 and j == 7)
                nc.tensor.matmul(scat[:, 0:280], st, msg[:, j, :],
                                 start=first, stop=last, skip_group_check=True)

        # ---- window finalize (keep att in f32 buffer; MLP deferred) ----
        f1 = sb_win.tile([128, CZ], FP32, tag="f1")
        recD = sb_win.tile([128, 16], FP32, tag="recD")
        att = attbuf[:, w, :]
        nc.vector.tensor_scalar(recD[:, 8:16], scat[:, 256:264], 1e-30, None, ALU.max)
        nc.vector.reciprocal(recD[:, 0:8], recD[:, 8:16])
        u1w = scat[:, 264:272].unsqueeze(1).broadcast_to([128, CO, 8])
        u2w = scat[:, 272:280].unsqueeze(1).broadcast_to([128, CO, 8])
        rD = recD[:, 0:8].unsqueeze(1).broadcast_to([128, CO, 8])
        nc.vector.tensor_mul(f1[:].rearrange("p (c h) -> p c h", h=8),
                             xcv[:].rearrange("p (c h) -> p c h", h=8), u1w)
        nc.vector.tensor_add(f1[:], scat[:, 0:256], f1[:])
        nc.vector.tensor_mul(att.rearrange("p (c h) -> p c h", h=8),
                             cs["svb"][:].rearrange("p (c h) -> p c h", h=8), u2w)
        nc.vector.tensor_sub(f1[:], f1[:], att)
        nc.vector.tensor_mul(f1[:].rearrange("p (c h) -> p c h", h=8),
                             f1[:].rearrange("p (c h) -> p c h", h=8), rD)
        nc.vector.tensor_add(att, f1[:], cs["cvb"][:])

    # ===== MLP tail: 2-window groups, stage-interleaved for engine overlap ====
    for w0 in range(0, NWIN_, 3):
        ws = list(range(w0, min(w0 + 3, NWIN_)))
        tps, at_ts, h1s, hsfs, hss, h_ts, yps = {}, {}, {}, {}, {}, {}, {}
        for w in ws:
            tp = p_q.tile([128, 2, 256], FP32, tag="q")
            for k in range(2):
                nc.tensor.transpose(tp[:, k, 0:128],
                                    attbuf[:, w, k * 128:(k + 1) * 128],
                                    cs["identf"][:])
            tps[w] = tp
        for w in ws:
            at_t = sb_mlp.tile([128, 2, 128], BF16, tag="at_t")
            nc.scalar.copy(at_t[:], tps[w][:, :, 0:128])
            at_ts[w] = at_t
        for w in ws:
            h1t = p_ea.tile([128, 2, 512], FP32, tag="ea")
            for k in range(2):
                nc.tensor.matmul(h1t[:, 0, :], at_ts[w][:, k, :], cs["w1"][:, k, :],
                                 start=(k == 0), stop=(k == 1))
            h1s[w] = h1t
        for w in ws:
            hsf = sb_mlp.tile([128, 512], FP32, tag="hsf")
            nc.vector.tensor_add(hsf[:], h1s[w][:, 0, :], cs["b1b"][:])
            hsfs[w] = hsf
        for w in ws:
            hs = sb_mlp.tile([128, 512], FP32, tag="hs")
            nc.scalar.activation(hs[:], hsfs[w][:], AF.Silu)
            hss[w] = hs
        for w in ws:
            hs = hss[w]
            tp2 = p_qd.tile([128, 2, 256], FP32, tag="qd")
            for k in range(2):
                nc.tensor.transpose(tp2[:, k, 0:128], hs[:, k * 128:(k + 1) * 128],
                                    cs["identf"][:])
            tp3 = p_q.tile([128, 2, 256], FP32, tag="q")
            for k in range(2):
                nc.tensor.transpose(tp3[:, k, 0:128],
                                    hs[:, (2 + k) * 128:(3 + k) * 128],
                                    cs["identf"][:])
            h_t = sb_mlp.tile([128, 4, 128], BF16, tag="h_t")
            nc.scalar.copy(h_t[:, 0:2, :], tp2[:, :, 0:128])
            nc.scalar.copy(h_t[:, 2:4, :], tp3[:, :, 0:128])
            h_ts[w] = h_t
        for w in ws:
            ypt = p_ea.tile([128, 2, 512], FP32, tag="ea")
            for k in range(4):
                nc.tensor.matmul(ypt[:, 0, 0:256], h_ts[w][:, k, :],
                                 cs["w2"][:, k, :],
                                 start=(k == 0), stop=(k == 3))
            yps[w] = ypt
        for w in ws:
            ys = sb_mlp.tile([128, CZ], FP32, tag="ys")
            nc.vector.tensor_add(ys[:], yps[w][:, 0, 0:256], cs["b2b"][:])
            nc.sync.dma_start(y_d[w * 128:(w + 1) * 128, :], ys[:])


_CACHE = {}


def kernel_ex(**inputs):
    cfg = Cfg(NWIN=NWIN)
    consts_h, in_maps, W_E, NSB, unshard = _host_prep(cfg, **inputs)
    cfg.W_E, cfg.NSB = W_E, NSB
    key = ("v3", W_E, NSB)
    if key not in _CACHE:
        nc = bacc.Bacc("TRN2", target_bir_lowering=False, debug=False,
                       num_devices=NCORES, num_swdge_queues=2)
        with tile.TileContext(nc, trace_sim=False) as tc:
            with ExitStack() as ctx:
                _build(nc, tc, ctx, consts_h, cfg)
        nc.compile()
        _CACHE[key] = nc
    nc = _CACHE[key]
    res = bass_utils.run_bass_kernel_spmd(nc, in_maps, core_ids=list(range(NCORES)))
    core_of, row_of = unshard
    ys = np.stack([res.results[c]["y"] for c in range(NCORES)])  # [8, 1280, 256]
    out = ys[core_of, row_of]
    return np.ascontiguousarray(out, np.float32), res


def kernel(**inputs):
    return kernel_ex(**inputs)[0]
